# revision 1
# baseline (speedup 1.0000x reference)
"""Trainium2 Bass kernel for nn_BatchedGaussianRenderer.

Math: each gaussian's per-pixel exponent is expanded as a 6-term polynomial
in centered pixel coordinates (x', y') = (x-63.5, y-63.5):

  expo(n, x, y) = f1*x'^2 + f2*x'y' + f3*y'^2 + f4*x' + f5*y' + f6
  image(x, y)   = sum_n exp(expo(n, x, y)),  then / max(image)

so the dense N x P evaluation is a K=6 matmul.  For fp32-grade accuracy on
the bf16 tensor engine, f and the pixel basis g are each split into 3 bf16
components and 6 cross products are kept (K=36, error ~2^-26 per term,
validated at ~2e-6 absmax-rel vs an fp64 oracle).

Sharding: each of the 8 cores computes ALL 4096 gaussians' coefficients
(cheap, ~100 vector ops on [128, blocks] layouts) and renders 16 image
rows (pixels x on partitions, gaussians streamed).  The ScalarEngine's
fused exp+row-sum (accum_out) produces the image directly; only a [1,1]
AllReduce(max) collective is needed for the final normalization.

The gaussians are processed in two halves so the second half's coefficient
computation (VectorEngine) overlaps the first half's rendering (ScalarE).

Per-gaussian preprocessing uses the unnormalized-quaternion fold: cov4D
scales uniformly by nsq = |q1|^2 |q2|^2, which cancels everywhere except
eps -> eps*nsq and inv_cov/lambda -> *nsq, avoiding rsqrt entirely.
sin/cos of the view angle are evaluated as Taylor polynomials on the DVE
(angle in [0,1)) so the only ACT table set ever loaded is exp's.
"""
import numpy as np
import ml_dtypes

import concourse.bass as bass
import concourse.bacc as bacc
import concourse.tile as tile
import concourse.mybir as mybir
from concourse import bass_utils

NG, H, W = 4096, 128, 128
ZOOM, EPS = 0.5, 1e-6
CX = CY = 63.5
SXY = (W - 1) / 2 * ZOOM          # 31.75
NCORES = 8
ROWS = H // NCORES                # 16 image rows per core
NB = NG // 128                    # 32 gaussian blocks (g = p*NB + b)
HB = NB // 2                      # 16 blocks per half
NSLOT = 6                         # (f-split, g-split) pairs
KP = NSLOT * 6                    # 36 K rows
dt = mybir.dt
AF = mybir.ActivationFunctionType
ALU = mybir.AluOpType

# ---------------------------------------------------------------- host helpers

def _bf16(x):
    return np.asarray(x, np.float32).astype(ml_dtypes.bfloat16).astype(np.float32)


def _g_lhsT_for_core(core):
    """[KP, ROWS*128] bf16 pixel-basis weights; see module docstring."""
    j_of_s = (0, 1, 2, 0, 1, 0)
    out = np.zeros((KP, ROWS * 128), np.float32)
    x = np.arange(128, dtype=np.float64) - CX
    for r in range(ROWS):
        y = ROWS * core + r - CY
        basis = np.stack([x * x, x * y, np.full(128, y * y), x,
                          np.full(128, y), np.ones(128)], 0)
        b32 = basis.astype(np.float32)
        g0 = _bf16(b32)
        g1 = _bf16(b32 - g0)
        g2 = _bf16(b32 - g0 - g1)
        gs = (g0, g1, g2)
        for s in range(NSLOT):
            for k in range(6):
                out[s * 6 + k, 128 * r:128 * (r + 1)] = gs[j_of_s[s]][k]
    return out.astype(ml_dtypes.bfloat16)


# L(q1) twisted copies: (out_off, out_stride, in_off, in_stride, count, sign),
# L stored per-block (i,k) slot = i*4+k, rotor comps a=(r0,r4,r5,r6).
L_COPIES = [
    (0, 1, 0, 1, 1, 1.0), (4, 4, 4, 1, 3, 1.0),
    (1, 12, 4, 1, 2, -1.0), (5, 4, 0, 6, 2, 1.0),
    (2, 4, 5, 1, 2, -1.0), (10, 4, 0, 4, 2, 1.0),
    (3, 1, 6, 1, 1, -1.0), (11, 1, 4, 1, 1, -1.0),
    (7, 1, 5, 1, 1, 1.0), (15, 1, 0, 1, 1, 1.0),
]
# R(conj q2) with q2 = (r7, -r1, -r2, -r3) folded; stored (j,k) slot = j*4+k.
R_COPIES = [
    (0, 1, 7, 1, 1, 1.0), (4, 4, 1, 1, 3, -1.0),
    (1, 4, 1, 6, 2, 1.0), (9, 1, 3, 1, 1, -1.0), (13, 1, 2, 1, 1, 1.0),
    (2, 4, 2, 1, 2, 1.0), (10, 1, 7, 1, 1, 1.0), (14, 1, 1, 1, 1, -1.0),
    (3, 1, 3, 1, 1, 1.0), (7, 1, 2, 1, 1, -1.0), (11, 4, 1, 6, 2, 1.0),
]

SIN_C = [1.0, -1.0 / 6, 1.0 / 120, -1.0 / 5040, 1.0 / 362880]      # of x^(2k+1)
COS_C = [1.0, -0.5, 1.0 / 24, -1.0 / 720, 1.0 / 40320, -1.0 / 3628800]


def build_nc():
    nc = bacc.Bacc("TRN2", target_bir_lowering=False, debug=False,
                   num_devices=NCORES)
    f32, bf16 = dt.float32, dt.bfloat16

    means_in = nc.dram_tensor("means", [NG, 4], f32, kind="ExternalInput").ap()
    raws_in = nc.dram_tensor("raw_scales", [NG, 4], f32, kind="ExternalInput").ap()
    rot_in = nc.dram_tensor("rotors", [NG, 8], f32, kind="ExternalInput").ap()
    t_in = nc.dram_tensor("t_scalar", [1, 1], f32, kind="ExternalInput").ap()
    ang_in = nc.dram_tensor("angle", [1, 1], f32, kind="ExternalInput").ap()
    g_in = nc.dram_tensor("g_lhsT", [KP, ROWS * 128], bf16, kind="ExternalInput").ap()
    idb_in = nc.dram_tensor("ident_bf", [128, 128], bf16, kind="ExternalInput").ap()
    idf_in = nc.dram_tensor("ident_f32", [128, 128], f32, kind="ExternalInput").ap()
    ones_in = nc.dram_tensor("ones_row", [1, 128], f32, kind="ExternalInput").ap()
    out_t = nc.dram_tensor("out", [ROWS, W], f32, kind="ExternalOutput").ap()

    with tile.TileContext(nc) as tc:
        with (
            tc.tile_pool(name="sb", bufs=1) as sb,
            tc.tile_pool(name="dram", bufs=1, space="DRAM") as dram,
        ):
            # ---------------- phase 0: loads + angle scalars ----------------
            MEANS = sb.tile([128, NB * 4], f32)
            RAWS = sb.tile([128, NB * 4], f32)
            ROT = sb.tile([128, NB * 8], f32)
            nc.sync.dma_start(MEANS[:], means_in.rearrange("(p b) c -> p (b c)", p=128))
            nc.sync.dma_start(RAWS[:], raws_in.rearrange("(p b) c -> p (b c)", p=128))
            nc.sync.dma_start(ROT[:], rot_in.rearrange("(p b) c -> p (b c)", p=128))
            G_SB = sb.tile([KP, ROWS * 128], bf16)
            nc.sync.dma_start(G_SB[:], g_in[:])
            IDB = sb.tile([128, 128], bf16)
            nc.sync.dma_start(IDB[:], idb_in[:])
            IDF = sb.tile([128, 128], f32)
            nc.sync.dma_start(IDF[:], idf_in[:])
            ONES = sb.tile([1, 128], f32)
            nc.sync.dma_start(ONES[:], ones_in[:])
            T_A = sb.tile([1, 1], f32)
            nc.sync.dma_start(T_A[:], t_in[:])
            ANG = sb.tile([1, 1], f32)
            nc.sync.dma_start(ANG[:], ang_in[:])


            # sin/cos via Taylor on DVE (angle in [0,1); no trig table load)
            U = sb.tile([1, 1], f32)
            SINA = sb.tile([1, 1], f32)
            COSA = sb.tile([1, 1], f32)
            nc.vector.tensor_mul(U[:], ANG[:], ANG[:])
            # cos/sin: Horner in u = x^2 (acc = acc*u + c per step)
            nc.vector.tensor_scalar(COSA[:], U[:], COS_C[5], COS_C[4],
                                    ALU.mult, ALU.add)
            for c in (COS_C[3], COS_C[2], COS_C[1], COS_C[0]):
                nc.vector.tensor_scalar(COSA[:], COSA[:], U[:], c,
                                        ALU.mult, ALU.add)
            nc.vector.tensor_scalar(SINA[:], U[:], SIN_C[4], SIN_C[3],
                                    ALU.mult, ALU.add)
            for c in (SIN_C[2], SIN_C[1], SIN_C[0]):
                nc.vector.tensor_scalar(SINA[:], SINA[:], U[:], c,
                                        ALU.mult, ALU.add)
            nc.vector.tensor_mul(SINA[:], SINA[:], ANG[:])

            # scalar vector: [sxc, sxs, A1, A2, A3, B1, B2, t]
            SCV = sb.tile([1, 8], f32)
            nc.vector.tensor_scalar_mul(SCV[:, 0:1], COSA[:], float(SXY))
            nc.vector.tensor_scalar_mul(SCV[:, 1:2], SINA[:], float(SXY))
            nc.vector.tensor_mul(SCV[:, 2:3], SCV[:, 0:1], SCV[:, 0:1])
            nc.vector.scalar_tensor_tensor(SCV[:, 3:4], SCV[:, 0:1], 2.0,
                                           SCV[:, 1:2], ALU.mult, ALU.mult)
            nc.vector.tensor_mul(SCV[:, 4:5], SCV[:, 1:2], SCV[:, 1:2])
            nc.vector.tensor_scalar_mul(SCV[:, 5:6], SCV[:, 0:1], float(SXY))
            nc.vector.tensor_scalar_mul(SCV[:, 6:7], SCV[:, 1:2], float(SXY))
            nc.vector.tensor_copy(SCV[:, 7:8], T_A[:])
            with tc.tile_pool(name="pp0", bufs=1, space="PSUM") as pp0:
                PBp = pp0.tile([128, 8], f32)
                nc.tensor.matmul(PBp[:], ONES[:], SCV[:], start=True, stop=True)
                SCB = sb.tile([128, 8], f32)
                nc.vector.tensor_copy(SCB[:], PBp[:])
            sxc_b, sxs_b = SCB[:, 0:1], SCB[:, 1:2]
            A1b, A2b, A3b = SCB[:, 2:3], SCB[:, 3:4], SCB[:, 4:5]
            B1b, B2b, tb = SCB[:, 5:6], SCB[:, 6:7], SCB[:, 7:8]

            means_h = [MEANS[:].rearrange("p (b c) -> p b c", c=4)]
            raws_h = [RAWS[:]]
            rot_h = [ROT[:].rearrange("p (b c) -> p b c", c=8)]

            def preprocess_half(h):
                """Emit coefficient computation for blocks [HB*h, HB*(h+1)).
                Returns the F36 tile [128, HB*36] bf16 (b, s, k)."""
                tg = lambda n: f"{n}{h}"
                B = NB
                S2 = sb.tile([128, B * 4], f32, tag=tg("S2"))
                nc.scalar.activation(S2[:], raws_h[h], AF.Exp, scale=2.0)

                SQ = sb.tile([128, B * 8], f32, tag=tg("SQ"))
                nc.vector.tensor_mul(SQ[:], rot_h[h], rot_h[h])
                sq = SQ[:].rearrange("p (b c) -> p b c", c=8)
                N1S = sb.tile([128, B], f32, tag=tg("N1S"))
                N2S = sb.tile([128, B], f32, tag=tg("N2S"))
                NSQ = sb.tile([128, B], f32, tag=tg("NSQ"))
                nc.vector.reduce_sum(N1S[:], sq[:, :, 4:7], axis=mybir.AxisListType.X)
                nc.vector.tensor_add(N1S[:], N1S[:], sq[:, :, 0])
                nc.vector.reduce_sum(N2S[:], sq[:, :, 1:4], axis=mybir.AxisListType.X)
                nc.vector.tensor_add(N2S[:], N2S[:], sq[:, :, 7])
                nc.vector.tensor_mul(NSQ[:], N1S[:], N2S[:])

                LT = sb.tile([128, B * 16], f32, tag=tg("LT"))
                RT = sb.tile([128, B * 16], f32, tag=tg("RT"))
                lt3 = LT[:].rearrange("p (b c) -> p b c", c=16)
                rt3 = RT[:].rearrange("p (b c) -> p b c", c=16)
                for dst, copies in ((lt3, L_COPIES), (rt3, R_COPIES)):
                    for (oo, os_, io, is_, cnt, sign) in copies:
                        out_ap = dst[:, :, oo::os_][:, :, :cnt] if cnt > 1 else dst[:, :, oo:oo + 1]
                        in_ap = rot_h[h][:, :, io::is_][:, :, :cnt] if cnt > 1 else rot_h[h][:, :, io:io + 1]
                        if sign > 0:
                            nc.gpsimd.tensor_copy(out_ap, in_ap)
                        else:
                            nc.scalar.mul(out_ap, in_ap, -1.0)

                P64 = sb.tile([128, B * 64], f32, tag=tg("P64"))
                lt4 = LT[:].rearrange("p (b i k) -> p b i k", i=4, k=4)
                rt4 = RT[:].rearrange("p (b j k) -> p b j k", j=4, k=4)
                p5 = P64[:].rearrange("p (b i j k) -> p b i j k", i=4, j=4, k=4)
                nc.vector.tensor_mul(
                    p5,
                    lt4.unsqueeze(3).broadcast_to([128, B, 4, 4, 4]),
                    rt4.unsqueeze(2).broadcast_to([128, B, 4, 4, 4]))
                R4 = sb.tile([128, B * 16], f32, tag=tg("R4"))
                nc.vector.reduce_sum(
                    R4[:], P64[:].rearrange("p (e k) -> p e k", k=4),
                    axis=mybir.AxisListType.X)

                M = sb.tile([128, B * 16], f32, tag=tg("M"))
                r44 = R4[:].rearrange("p (b i j) -> p b i j", i=4, j=4)
                s23 = S2[:].rearrange("p (b c) -> p b c", c=4)
                m4 = M[:].rearrange("p (b i j) -> p b i j", i=4, j=4)
                nc.vector.tensor_mul(
                    m4, r44, s23.unsqueeze(2).broadcast_to([128, B, 4, 4]))
                C64 = sb.tile([128, B * 64], f32, tag=tg("C64"))
                c5 = C64[:].rearrange("p (b i k j) -> p b i k j", i=4, k=4, j=4)
                nc.vector.tensor_mul(
                    c5,
                    m4.unsqueeze(3).broadcast_to([128, B, 4, 4, 4]),
                    r44.unsqueeze(2).broadcast_to([128, B, 4, 4, 4]))
                C16 = sb.tile([128, B * 16], f32, tag=tg("C16"))
                nc.vector.reduce_sum(
                    C16[:], C64[:].rearrange("p (e j) -> p e j", j=4),
                    axis=mybir.AxisListType.X)
                c16 = C16[:].rearrange("p (b e) -> p b e", e=16)

                EPN = sb.tile([128, B], f32, tag=tg("EPN"))
                nc.vector.tensor_scalar_mul(EPN[:], NSQ[:], float(EPS))
                WP = sb.tile([128, B], f32, tag=tg("WP"))
                nc.vector.tensor_max(WP[:], c16[:, :, 15], EPN[:])
                IW = sb.tile([128, B], f32, tag=tg("IW"))
                nc.vector.reciprocal(IW[:], WP[:])
                TD = sb.tile([128, B], f32, tag=tg("TD"))
                nc.scalar.activation(TD[:], means_h[h][:, :, 3], AF.Identity,
                                     bias=tb, scale=-1.0)
                TDW = sb.tile([128, B], f32, tag=tg("TDW"))
                nc.vector.tensor_mul(TDW[:], TD[:], IW[:])
                W1 = sb.tile([128, B], f32, tag=tg("W1"))
                nc.vector.tensor_mul(W1[:], NSQ[:], IW[:])
                Z3 = sb.tile([128, B], f32, tag=tg("Z3"))
                nc.vector.tensor_mul(Z3[:], W1[:], TD[:])

                VV9 = sb.tile([128, B * 9], f32, tag=tg("VV9"))
                vv3 = VV9[:].rearrange("p (b i k) -> p b i k", i=3, k=3)
                v_i = c16[:, :, 3::4][:, :, 0:3]
                nc.vector.tensor_mul(
                    vv3,
                    v_i.unsqueeze(3).broadcast_to([128, B, 3, 3]),
                    v_i.unsqueeze(2).broadcast_to([128, B, 3, 3]))
                CV3 = sb.tile([128, B * 9], f32, tag=tg("CV3"))
                cv3f = CV3[:].rearrange("p (b e) -> p b e", e=9)
                iw_b9 = IW[:].unsqueeze(2).broadcast_to([128, B, 9])
                vv9f = VV9[:].rearrange("p (b e) -> p b e", e=9)
                nc.vector.tensor_mul(cv3f, vv9f, iw_b9)
                u9 = c16.rearrange("p b (i k) -> p b i k", i=4)[:, :, 0:3, 0:3]
                cv33 = CV3[:].rearrange("p (b i k) -> p b i k", i=3, k=3)
                nc.vector.tensor_sub(cv33, u9, cv33)

                MU3 = sb.tile([128, B * 3], f32, tag=tg("MU3"))
                mu33 = MU3[:].rearrange("p (b c) -> p b c", c=3)
                tdw_b3 = TDW[:].unsqueeze(2).broadcast_to([128, B, 3])
                nc.vector.tensor_mul(mu33, v_i, tdw_b3)
                nc.vector.tensor_add(mu33, mu33, means_h[h][:, :, 0:3])

                MX = sb.tile([128, B], f32, tag=tg("MX"))
                MY = sb.tile([128, B], f32, tag=tg("MY"))
                TMP = sb.tile([128, B], f32, tag=tg("TMP"))
                TMP2 = sb.tile([128, B], f32, tag=tg("TMP2"))
                nc.vector.tensor_scalar(TMP[:], mu33[:, :, 2], sxs_b, None, ALU.mult)
                nc.vector.scalar_tensor_tensor(MX[:], mu33[:, :, 0], sxc_b, TMP[:],
                                               ALU.mult, ALU.add)
                nc.vector.tensor_scalar_mul(MY[:], mu33[:, :, 1], float(SXY))

                cv3e = CV3[:].rearrange("p (b e) -> p b e", e=9)
                AE = sb.tile([128, B], f32, tag=tg("AE"))
                BE = sb.tile([128, B], f32, tag=tg("BE"))
                DE = sb.tile([128, B], f32, tag=tg("DE"))
                nc.vector.tensor_scalar(TMP[:], cv3e[:, :, 8], A3b, None, ALU.mult)
                nc.vector.scalar_tensor_tensor(TMP[:], cv3e[:, :, 2], A2b, TMP[:],
                                               ALU.mult, ALU.add)
                nc.vector.scalar_tensor_tensor(AE[:], cv3e[:, :, 0], A1b, TMP[:],
                                               ALU.mult, ALU.add)
                nc.vector.tensor_add(AE[:], AE[:], EPN[:])
                nc.vector.tensor_scalar(TMP[:], cv3e[:, :, 5], B2b, None, ALU.mult)
                nc.vector.scalar_tensor_tensor(BE[:], cv3e[:, :, 1], B1b, TMP[:],
                                               ALU.mult, ALU.add)
                nc.vector.tensor_scalar_mul(DE[:], cv3e[:, :, 4], float(SXY * SXY))
                nc.vector.tensor_add(DE[:], DE[:], EPN[:])

                DET = sb.tile([128, B], f32, tag=tg("DET"))
                nc.vector.tensor_mul(DET[:], AE[:], DE[:])
                nc.vector.tensor_mul(TMP[:], BE[:], BE[:])
                nc.vector.tensor_sub(DET[:], DET[:], TMP[:])
                IDN = sb.tile([128, B], f32, tag=tg("IDN"))
                nc.vector.reciprocal(IDN[:], DET[:])
                nc.vector.tensor_mul(IDN[:], IDN[:], NSQ[:])
                IA = sb.tile([128, B], f32, tag=tg("IA"))
                ID_ = sb.tile([128, B], f32, tag=tg("ID_"))
                F2T = sb.tile([128, B], f32, tag=tg("F2T"))
                nc.vector.tensor_mul(IA[:], DE[:], IDN[:])
                nc.vector.tensor_mul(F2T[:], BE[:], IDN[:])
                nc.vector.tensor_mul(ID_[:], AE[:], IDN[:])

                F6 = sb.tile([128, B * 6], f32, tag=tg("F6"))
                f63 = F6[:].rearrange("p (b k) -> p b k", k=6)
                nc.vector.tensor_scalar_mul(f63[:, :, 0], IA[:], -0.5)
                nc.vector.tensor_copy(f63[:, :, 1], F2T[:])
                nc.vector.tensor_scalar_mul(f63[:, :, 2], ID_[:], -0.5)
                nc.vector.tensor_mul(TMP[:], IA[:], MX[:])
                nc.vector.tensor_mul(TMP2[:], F2T[:], MY[:])
                nc.vector.tensor_sub(f63[:, :, 3], TMP[:], TMP2[:])
                nc.vector.tensor_mul(TMP[:], ID_[:], MY[:])
                nc.vector.tensor_mul(TMP2[:], F2T[:], MX[:])
                nc.vector.tensor_sub(f63[:, :, 4], TMP[:], TMP2[:])
                nc.vector.tensor_mul(TMP[:], MX[:], f63[:, :, 3])
                nc.vector.tensor_mul(TMP2[:], MY[:], f63[:, :, 4])
                nc.vector.tensor_add(TMP[:], TMP[:], TMP2[:])
                nc.vector.tensor_mul(TMP2[:], Z3[:], TD[:])
                nc.vector.tensor_add(TMP[:], TMP[:], TMP2[:])
                nc.vector.tensor_scalar_mul(f63[:, :, 5], TMP[:], -0.5)

                F36 = sb.tile([128, B * KP], bf16, tag=tg("F36"))
                f364 = F36[:].rearrange("p (b s k) -> p b s k", s=NSLOT, k=6)
                R1 = sb.tile([128, B * 6], f32, tag=tg("R1"))
                R2 = sb.tile([128, B * 6], f32, tag=tg("R2"))
                r13 = R1[:].rearrange("p (b k) -> p b k", k=6)
                r23 = R2[:].rearrange("p (b k) -> p b k", k=6)
                nc.gpsimd.tensor_copy(f364[:, :, 0, :], f63)
                nc.gpsimd.tensor_copy(f364[:, :, 1, :], f364[:, :, 0, :])
                nc.gpsimd.tensor_copy(f364[:, :, 2, :], f364[:, :, 0, :])
                nc.vector.tensor_sub(r13, f63, f364[:, :, 0, :])
                nc.gpsimd.tensor_copy(f364[:, :, 3, :], r13)
                nc.gpsimd.tensor_copy(f364[:, :, 4, :], f364[:, :, 3, :])
                nc.vector.tensor_sub(r23, r13, f364[:, :, 3, :])
                nc.gpsimd.tensor_copy(f364[:, :, 5, :], r23)
                return F36

            F36_0 = preprocess_half(0)
            FS = [sb.tile([KP, NG // 2], bf16, tag=f"FS{h}", name=f"FS{h}")
                  for h in range(2)]

            ACC = sb.tile([128, 2 * ROWS], f32)
            IMG = sb.tile([128, ROWS], f32)
            RMX = sb.tile([128, 1], f32)

            def transpose_half(h, F36h):
                for q in range(HB // 4):
                    TP = dp.tile([KP, 512], bf16, tag="pt")
                    for c in range(4):
                        b = HB * h + 4 * q + c
                        nc.tensor.transpose(TP[:, 128 * c:128 * (c + 1)],
                                            F36h[:, KP * b:KP * (b + 1)], IDB[:])
                    eng = nc.scalar if h == 0 else nc.vector
                    eng_copy = (nc.scalar.copy if h == 0 else
                                (lambda o, i: nc.vector.tensor_copy(o, i)))
                    eng_copy(FS[h][:, 512 * q:512 * (q + 1)], TP[:])

            def dense_half(h2):
                for r in range(ROWS):
                    PT = dp.tile([128, 2048], dt.float32, tag="pt")
                    for s in range(4):
                        nc.tensor.matmul(
                            PT[:, 512 * s:512 * (s + 1)],
                            G_SB[:, 128 * r:128 * (r + 1)],
                            FS[h2][:, 512 * s:512 * (s + 1)],
                            start=True, stop=True)
                    col = 2 * r + h2
                    nc.scalar.activation(PT[:], PT[:], AF.Exp,
                                         accum_out=ACC[:, col:col + 1])

            with tc.tile_pool(name="dp", bufs=2, space="PSUM") as dp:
                transpose_half(0, F36_0)
                dense_half(0)
                transpose_half(1, F36_0)
                dense_half(1)

            acc3 = ACC[:].rearrange("p (r h) -> p r h", h=2)
            nc.vector.tensor_add(IMG[:], acc3[:, :, 0], acc3[:, :, 1])
            nc.vector.reduce_max(RMX[:], IMG[:], axis=mybir.AxisListType.X)

            # ------- phase 3: global max (AllReduce) + normalize -------
            with tc.tile_pool(name="tp", bufs=1, space="PSUM") as tp:
                RMTp = tp.tile([1, 128], dt.float32)
                nc.tensor.transpose(RMTp[:], RMX[:], IDF[:])
                RMT = sb.tile([1, 128], dt.float32)
                nc.vector.tensor_copy(RMT[:], RMTp[:])
                LMAX = sb.tile([1, 1], dt.float32)
                nc.vector.reduce_max(LMAX[:], RMT[:], axis=mybir.AxisListType.X)
                cin = dram.tile([1, 1], dt.float32)
                cout = dram.tile([1, 1], dt.float32)
                nc.sync.dma_start(cin[:], LMAX[:])
                nc.gpsimd.collective_compute(
                    "AllReduce", ALU.max,
                    replica_groups=[list(range(NCORES))],
                    ins=[cin[:].opt()], outs=[cout[:].opt()])
                GM = sb.tile([1, 1], dt.float32)
                nc.sync.dma_start(GM[:], cout[:])
                nc.vector.tensor_scalar_max(GM[:], GM[:], float(EPS))
                RI = sb.tile([1, 1], dt.float32)
                nc.vector.reciprocal(RI[:], GM[:])
                RIBp = tp.tile([128, 1], dt.float32)
                nc.tensor.matmul(RIBp[:], ONES[:], RI[:], start=True, stop=True)
                RIB = sb.tile([128, 1], dt.float32)
                nc.vector.tensor_copy(RIB[:], RIBp[:])
                nc.vector.tensor_scalar(IMG[:], IMG[:], RIB[:], None, ALU.mult)

                OTp = tp.tile([ROWS, 128], dt.float32)
                nc.tensor.transpose(OTp[:], IMG[:], IDF[:])
                OT = sb.tile([ROWS, 128], dt.float32)
                nc.vector.tensor_copy(OT[:], OTp[:])
                nc.sync.dma_start(out_t[:], OT[:])

    nc.compile()
    return nc


_NC_CACHE = {}


def _get_nc():
    if "nc" not in _NC_CACHE:
        _NC_CACHE["nc"] = build_nc()
    return _NC_CACHE["nc"]


def _make_in_maps(means, raw_scales, rotors, t, angle):
    means = np.ascontiguousarray(np.asarray(means, np.float32))
    raw_scales = np.ascontiguousarray(np.asarray(raw_scales, np.float32))
    rotors = np.ascontiguousarray(np.asarray(rotors, np.float32))
    t_arr = np.array([[np.float32(t)]], np.float32)
    a_arr = np.array([[np.float32(angle)]], np.float32)
    idb = np.eye(128, dtype=np.float32).astype(ml_dtypes.bfloat16)
    idf = np.eye(128, dtype=np.float32)
    ones = np.ones((1, 128), np.float32)
    in_maps = []
    for c in range(NCORES):
        in_maps.append({
            "means": means, "raw_scales": raw_scales, "rotors": rotors,
            "t_scalar": t_arr, "angle": a_arr,
            "g_lhsT": np.ascontiguousarray(_g_lhsT_for_core(c)),
            "ident_bf": idb, "ident_f32": idf, "ones_row": ones,
        })
    return in_maps


def run(means, raw_scales, rotors, t, angle, trace=False):
    """Returns (image [128,128] fp32, BassKernelResults)."""
    nc = _get_nc()
    in_maps = _make_in_maps(means, raw_scales, rotors, t, angle)
    res = bass_utils.run_bass_kernel_spmd(
        nc, in_maps, core_ids=list(range(NCORES)), trace=trace)
    img = np.concatenate([res.results[c]["out"] for c in range(NCORES)], axis=0)
    return img.astype(np.float32), res


def kernel(**inputs):
    img, _ = run(inputs["means"], inputs["raw_scales"], inputs["rotors"],
                 inputs["t"], inputs["angle"])
    return img



# revision 4
# speedup vs baseline: 1.0048x; 1.0048x over previous
"""Trainium2 Bass kernel for nn_BatchedGaussianRenderer.

Math: each gaussian's per-pixel exponent is expanded as a 6-term polynomial
in centered pixel coordinates (x', y') = (x-63.5, y-63.5):

  expo(n, x, y) = f1*x'^2 + f2*x'y' + f3*y'^2 + f4*x' + f5*y' + f6
  image(x, y)   = sum_n exp(expo(n, x, y)),  then / max(image)

so the dense N x P evaluation is a K=6 matmul.  For accuracy on the bf16
tensor engine, f and the pixel basis g are each split into 2 bf16
components and the 3 leading cross products kept (K=18, error ~2^-17 per
term, validated at ~3e-6 absmax-rel vs an fp64 oracle).

Sharding: each of the 8 cores computes ALL 4096 gaussians' coefficients
(cheap, ~100 vector ops on [128, blocks] layouts) and renders 16 image
rows (pixels x on partitions, gaussians streamed).  K=18 <= 32 lets the
dense matmuls be packed 4-to-the-PE-array via tile_position row groups:
per image row, 4 concurrent [18x128]x[18x512] matmuls cover 2048
gaussians in ~512 PE cycles.  The ScalarEngine's fused exp+row-sum
(accum_out) produces the image directly; an 8-value AllGather + local max
replaces the AllReduce for the final normalization.

The gaussians are processed in two halves so the second half's coefficient
computation (VectorEngine) overlaps the first half's rendering (ScalarE).

Per-gaussian preprocessing uses the unnormalized-quaternion fold: cov4D
scales uniformly by nsq = |q1|^2 |q2|^2, which cancels everywhere except
eps -> eps*nsq and inv_cov/lambda -> *nsq, avoiding rsqrt entirely.
sin/cos of the view angle are evaluated as Taylor polynomials on the DVE
(angle in [0,1)) so the only ACT table set ever loaded is exp's.
"""
import numpy as np
import ml_dtypes

import concourse.bass as bass
import concourse.bacc as bacc
import concourse.tile as tile
import concourse.mybir as mybir
from concourse import bass_utils

NG, H, W = 4096, 128, 128
ZOOM, EPS = 0.5, 1e-6
CX = CY = 63.5
SXY = (W - 1) / 2 * ZOOM          # 31.75
NCORES = 8
ROWS = H // NCORES                # 16 image rows per core
NB = NG // 128                    # 32 gaussian blocks (g = p*NB + b)
HB = NB // 2                      # 16 blocks per half
NSLOT = 3                         # (f-split, g-split) pairs: 00, 01, 10
KP = NSLOT * 6                    # 18 K rows
dt = mybir.dt
AF = mybir.ActivationFunctionType
ALU = mybir.AluOpType

# ---------------------------------------------------------------- host helpers

def _bf16(x):
    return np.asarray(x, np.float32).astype(ml_dtypes.bfloat16).astype(np.float32)


def _g_lhsT_for_core(core):
    """[128, ROWS*128] bf16 pixel-basis weights, replicated into the four
    32-partition groups (rows 32i..32i+17 identical) for tile_position
    row-group packing; see module docstring."""
    jg_of_s = (0, 1, 0)           # g-split component per slot
    out = np.zeros((128, ROWS * 128), np.float32)
    x = np.arange(128, dtype=np.float64) - CX
    for r in range(ROWS):
        y = ROWS * core + r - CY
        basis = np.stack([x * x, x * y, np.full(128, y * y), x,
                          np.full(128, y), np.ones(128)], 0)
        b32 = basis.astype(np.float32)
        g0 = _bf16(b32)
        g1 = _bf16(b32 - g0)
        gs = (g0, g1)
        for s in range(NSLOT):
            for k in range(6):
                row = gs[jg_of_s[s]][k]
                for i in range(4):
                    out[32 * i + s * 6 + k, 128 * r:128 * (r + 1)] = row
    return out.astype(ml_dtypes.bfloat16)


# L(q1) twisted copies: (out_off, out_stride, in_off, in_stride, count, sign),
# L stored per-block (i,k) slot = i*4+k, rotor comps a=(r0,r4,r5,r6).
L_COPIES = [
    (0, 1, 0, 1, 1, 1.0), (4, 4, 4, 1, 3, 1.0),
    (1, 12, 4, 1, 2, -1.0), (5, 4, 0, 6, 2, 1.0),
    (2, 4, 5, 1, 2, -1.0), (10, 4, 0, 4, 2, 1.0),
    (3, 1, 6, 1, 1, -1.0), (11, 1, 4, 1, 1, -1.0),
    (7, 1, 5, 1, 1, 1.0), (15, 1, 0, 1, 1, 1.0),
]
# R(conj q2) with q2 = (r7, -r1, -r2, -r3) folded; stored (j,k) slot = j*4+k.
R_COPIES = [
    (0, 1, 7, 1, 1, 1.0), (4, 4, 1, 1, 3, -1.0),
    (1, 4, 1, 6, 2, 1.0), (9, 1, 3, 1, 1, -1.0), (13, 1, 2, 1, 1, 1.0),
    (2, 4, 2, 1, 2, 1.0), (10, 1, 7, 1, 1, 1.0), (14, 1, 1, 1, 1, -1.0),
    (3, 1, 3, 1, 1, 1.0), (7, 1, 2, 1, 1, -1.0), (11, 4, 1, 6, 2, 1.0),
]

SIN_C = [1.0, -1.0 / 6, 1.0 / 120, -1.0 / 5040, 1.0 / 362880]      # of x^(2k+1)
COS_C = [1.0, -0.5, 1.0 / 24, -1.0 / 720, 1.0 / 40320, -1.0 / 3628800]


def build_nc():
    nc = bacc.Bacc("TRN2", target_bir_lowering=False, debug=False,
                   num_devices=NCORES)
    f32, bf16 = dt.float32, dt.bfloat16

    means_in = nc.dram_tensor("means", [NG, 4], f32, kind="ExternalInput").ap()
    raws_in = nc.dram_tensor("raw_scales", [NG, 4], f32, kind="ExternalInput").ap()
    rot_in = nc.dram_tensor("rotors", [NG, 8], f32, kind="ExternalInput").ap()
    t_in = nc.dram_tensor("t_scalar", [1, 1], f32, kind="ExternalInput").ap()
    ang_in = nc.dram_tensor("angle", [1, 1], f32, kind="ExternalInput").ap()
    g_in = nc.dram_tensor("g_lhsT", [128, ROWS * 128], bf16, kind="ExternalInput").ap()
    idb_in = nc.dram_tensor("ident_bf", [128, 128], bf16, kind="ExternalInput").ap()
    idf_in = nc.dram_tensor("ident_f32", [128, 128], f32, kind="ExternalInput").ap()
    ones_in = nc.dram_tensor("ones_row", [1, 128], f32, kind="ExternalInput").ap()
    out_t = nc.dram_tensor("out", [ROWS, W], f32, kind="ExternalOutput").ap()

    with tile.TileContext(nc) as tc:
        with (
            tc.tile_pool(name="sb", bufs=1) as sb,
            tc.tile_pool(name="dram", bufs=1, space="DRAM") as dram,
        ):
            # ---------------- phase 0: loads + angle scalars ----------------
            MEANS = sb.tile([128, NB * 4], f32)
            RAWS = sb.tile([128, NB * 4], f32)
            ROT = sb.tile([128, NB * 8], f32)
            nc.sync.dma_start(MEANS[:], means_in.rearrange("(p b) c -> p (b c)", p=128))
            nc.sync.dma_start(RAWS[:], raws_in.rearrange("(p b) c -> p (b c)", p=128))
            nc.sync.dma_start(ROT[:], rot_in.rearrange("(p b) c -> p (b c)", p=128))
            G_SB = sb.tile([128, ROWS * 128], bf16)
            nc.sync.dma_start(G_SB[:], g_in[:])
            IDB = sb.tile([128, 128], bf16)
            nc.sync.dma_start(IDB[:], idb_in[:])
            IDF = sb.tile([128, 128], f32)
            nc.sync.dma_start(IDF[:], idf_in[:])
            ONES = sb.tile([1, 128], f32)
            nc.sync.dma_start(ONES[:], ones_in[:])
            T_A = sb.tile([1, 1], f32)
            nc.sync.dma_start(T_A[:], t_in[:])
            ANG = sb.tile([1, 1], f32)
            nc.sync.dma_start(ANG[:], ang_in[:])

            # sin/cos via Taylor on DVE (angle in [0,1); no trig table load)
            U = sb.tile([1, 1], f32)
            SINA = sb.tile([1, 1], f32)
            COSA = sb.tile([1, 1], f32)
            nc.vector.tensor_mul(U[:], ANG[:], ANG[:])
            # cos/sin: Horner in u = x^2 (acc = acc*u + c per step)
            nc.vector.tensor_scalar(COSA[:], U[:], COS_C[5], COS_C[4],
                                    ALU.mult, ALU.add)
            for c in (COS_C[3], COS_C[2], COS_C[1], COS_C[0]):
                nc.vector.tensor_scalar(COSA[:], COSA[:], U[:], c,
                                        ALU.mult, ALU.add)
            nc.vector.tensor_scalar(SINA[:], U[:], SIN_C[4], SIN_C[3],
                                    ALU.mult, ALU.add)
            for c in (SIN_C[2], SIN_C[1], SIN_C[0]):
                nc.vector.tensor_scalar(SINA[:], SINA[:], U[:], c,
                                        ALU.mult, ALU.add)
            nc.vector.tensor_mul(SINA[:], SINA[:], ANG[:])

            # scalar vector: [sxc, sxs, A1, A2, A3, B1, B2, t]
            SCV = sb.tile([1, 8], f32)
            nc.vector.tensor_scalar_mul(SCV[:, 0:1], COSA[:], float(SXY))
            nc.vector.tensor_scalar_mul(SCV[:, 1:2], SINA[:], float(SXY))
            nc.vector.tensor_mul(SCV[:, 2:3], SCV[:, 0:1], SCV[:, 0:1])
            nc.vector.scalar_tensor_tensor(SCV[:, 3:4], SCV[:, 0:1], 2.0,
                                           SCV[:, 1:2], ALU.mult, ALU.mult)
            nc.vector.tensor_mul(SCV[:, 4:5], SCV[:, 1:2], SCV[:, 1:2])
            nc.vector.tensor_scalar_mul(SCV[:, 5:6], SCV[:, 0:1], float(SXY))
            nc.vector.tensor_scalar_mul(SCV[:, 6:7], SCV[:, 1:2], float(SXY))
            nc.vector.tensor_copy(SCV[:, 7:8], T_A[:])
            with tc.tile_pool(name="pp0", bufs=1, space="PSUM") as pp0:
                PBp = pp0.tile([128, 8], f32)
                nc.tensor.matmul(PBp[:], ONES[:], SCV[:], start=True, stop=True)
                SCB = sb.tile([128, 8], f32)
                nc.vector.tensor_copy(SCB[:], PBp[:])
            sxc_b, sxs_b = SCB[:, 0:1], SCB[:, 1:2]
            A1b, A2b, A3b = SCB[:, 2:3], SCB[:, 3:4], SCB[:, 4:5]
            B1b, B2b, tb = SCB[:, 5:6], SCB[:, 6:7], SCB[:, 7:8]

            means_a = MEANS[:].rearrange("p (b c) -> p b c", c=4)
            rot_a = ROT[:].rearrange("p (b c) -> p b c", c=8)

            def preprocess_half(h):
                """Emit coefficient computation for blocks [HB*h, HB*(h+1)).
                Returns the F18 tile [128, HB*18] bf16 (b, s, k)."""
                tg = lambda n: f"{n}{h}"
                B = HB
                means_h = means_a[:, HB * h:HB * (h + 1), :]
                rot_h = rot_a[:, HB * h:HB * (h + 1), :]
                raws_h = RAWS[:, HB * 4 * h:HB * 4 * (h + 1)]

                S2 = sb.tile([128, B * 4], f32, tag=tg("S2"))
                nc.scalar.activation(S2[:], raws_h, AF.Exp, scale=2.0)

                SQ = sb.tile([128, B * 8], f32, tag=tg("SQ"))
                nc.vector.tensor_mul(SQ[:], rot_h, rot_h)
                sq = SQ[:].rearrange("p (b c) -> p b c", c=8)
                N1S = sb.tile([128, B], f32, tag=tg("N1S"))
                N2S = sb.tile([128, B], f32, tag=tg("N2S"))
                NSQ = sb.tile([128, B], f32, tag=tg("NSQ"))
                nc.vector.reduce_sum(N1S[:], sq[:, :, 4:7], axis=mybir.AxisListType.X)
                nc.vector.tensor_add(N1S[:], N1S[:], sq[:, :, 0])
                nc.vector.reduce_sum(N2S[:], sq[:, :, 1:4], axis=mybir.AxisListType.X)
                nc.vector.tensor_add(N2S[:], N2S[:], sq[:, :, 7])
                nc.vector.tensor_mul(NSQ[:], N1S[:], N2S[:])

                LT = sb.tile([128, B * 16], f32, tag=tg("LT"))
                RT = sb.tile([128, B * 16], f32, tag=tg("RT"))
                lt3 = LT[:].rearrange("p (b c) -> p b c", c=16)
                rt3 = RT[:].rearrange("p (b c) -> p b c", c=16)
                for dst, copies in ((lt3, L_COPIES), (rt3, R_COPIES)):
                    for (oo, os_, io, is_, cnt, sign) in copies:
                        out_ap = dst[:, :, oo::os_][:, :, :cnt] if cnt > 1 else dst[:, :, oo:oo + 1]
                        in_ap = rot_h[:, :, io::is_][:, :, :cnt] if cnt > 1 else rot_h[:, :, io:io + 1]
                        if sign > 0:
                            nc.gpsimd.tensor_copy(out_ap, in_ap)
                        else:
                            nc.gpsimd.tensor_scalar_mul(out_ap, in_ap, -1.0)

                P64 = sb.tile([128, B * 64], f32, tag=tg("P64"))
                lt4 = LT[:].rearrange("p (b i k) -> p b i k", i=4, k=4)
                rt4 = RT[:].rearrange("p (b j k) -> p b j k", j=4, k=4)
                p5 = P64[:].rearrange("p (b i j k) -> p b i j k", i=4, j=4, k=4)
                nc.vector.tensor_mul(
                    p5,
                    lt4.unsqueeze(3).broadcast_to([128, B, 4, 4, 4]),
                    rt4.unsqueeze(2).broadcast_to([128, B, 4, 4, 4]))
                R4 = sb.tile([128, B * 16], f32, tag=tg("R4"))
                nc.vector.reduce_sum(
                    R4[:], P64[:].rearrange("p (e k) -> p e k", k=4),
                    axis=mybir.AxisListType.X)

                M = sb.tile([128, B * 16], f32, tag=tg("M"))
                r44 = R4[:].rearrange("p (b i j) -> p b i j", i=4, j=4)
                s23 = S2[:].rearrange("p (b c) -> p b c", c=4)
                m4 = M[:].rearrange("p (b i j) -> p b i j", i=4, j=4)
                nc.vector.tensor_mul(
                    m4, r44, s23.unsqueeze(2).broadcast_to([128, B, 4, 4]))
                C64 = sb.tile([128, B * 64], f32, tag=tg("C64"))
                c5 = C64[:].rearrange("p (b i k j) -> p b i k j", i=4, k=4, j=4)
                nc.vector.tensor_mul(
                    c5,
                    m4.unsqueeze(3).broadcast_to([128, B, 4, 4, 4]),
                    r44.unsqueeze(2).broadcast_to([128, B, 4, 4, 4]))
                C16 = sb.tile([128, B * 16], f32, tag=tg("C16"))
                nc.vector.reduce_sum(
                    C16[:], C64[:].rearrange("p (e j) -> p e j", j=4),
                    axis=mybir.AxisListType.X)
                c16 = C16[:].rearrange("p (b e) -> p b e", e=16)

                EPN = sb.tile([128, B], f32, tag=tg("EPN"))
                nc.vector.tensor_scalar_mul(EPN[:], NSQ[:], float(EPS))
                WP = sb.tile([128, B], f32, tag=tg("WP"))
                nc.vector.tensor_max(WP[:], c16[:, :, 15], EPN[:])
                IW = sb.tile([128, B], f32, tag=tg("IW"))
                nc.vector.reciprocal(IW[:], WP[:])
                TD = sb.tile([128, B], f32, tag=tg("TD"))
                nc.scalar.activation(TD[:], means_h[:, :, 3], AF.Identity,
                                     bias=tb, scale=-1.0)
                TDW = sb.tile([128, B], f32, tag=tg("TDW"))
                nc.vector.tensor_mul(TDW[:], TD[:], IW[:])
                W1 = sb.tile([128, B], f32, tag=tg("W1"))
                nc.vector.tensor_mul(W1[:], NSQ[:], IW[:])
                Z3 = sb.tile([128, B], f32, tag=tg("Z3"))
                nc.vector.tensor_mul(Z3[:], W1[:], TD[:])

                VV9 = sb.tile([128, B * 9], f32, tag=tg("VV9"))
                vv3 = VV9[:].rearrange("p (b i k) -> p b i k", i=3, k=3)
                v_i = c16[:, :, 3::4][:, :, 0:3]
                nc.vector.tensor_mul(
                    vv3,
                    v_i.unsqueeze(3).broadcast_to([128, B, 3, 3]),
                    v_i.unsqueeze(2).broadcast_to([128, B, 3, 3]))
                CV3 = sb.tile([128, B * 9], f32, tag=tg("CV3"))
                cv3f = CV3[:].rearrange("p (b e) -> p b e", e=9)
                iw_b9 = IW[:].unsqueeze(2).broadcast_to([128, B, 9])
                vv9f = VV9[:].rearrange("p (b e) -> p b e", e=9)
                nc.vector.tensor_mul(cv3f, vv9f, iw_b9)
                u9 = c16.rearrange("p b (i k) -> p b i k", i=4)[:, :, 0:3, 0:3]
                cv33 = CV3[:].rearrange("p (b i k) -> p b i k", i=3, k=3)
                nc.vector.tensor_sub(cv33, u9, cv33)

                MU3 = sb.tile([128, B * 3], f32, tag=tg("MU3"))
                mu33 = MU3[:].rearrange("p (b c) -> p b c", c=3)
                tdw_b3 = TDW[:].unsqueeze(2).broadcast_to([128, B, 3])
                nc.vector.tensor_mul(mu33, v_i, tdw_b3)
                nc.vector.tensor_add(mu33, mu33, means_h[:, :, 0:3])

                MX = sb.tile([128, B], f32, tag=tg("MX"))
                MY = sb.tile([128, B], f32, tag=tg("MY"))
                TMP = sb.tile([128, B], f32, tag=tg("TMP"))
                TMP2 = sb.tile([128, B], f32, tag=tg("TMP2"))
                nc.vector.tensor_scalar(TMP[:], mu33[:, :, 2], sxs_b, None, ALU.mult)
                nc.vector.scalar_tensor_tensor(MX[:], mu33[:, :, 0], sxc_b, TMP[:],
                                               ALU.mult, ALU.add)
                nc.vector.tensor_scalar_mul(MY[:], mu33[:, :, 1], float(SXY))

                cv3e = CV3[:].rearrange("p (b e) -> p b e", e=9)
                AE = sb.tile([128, B], f32, tag=tg("AE"))
                BE = sb.tile([128, B], f32, tag=tg("BE"))
                DE = sb.tile([128, B], f32, tag=tg("DE"))
                nc.vector.tensor_scalar(TMP[:], cv3e[:, :, 8], A3b, None, ALU.mult)
                nc.vector.scalar_tensor_tensor(TMP[:], cv3e[:, :, 2], A2b, TMP[:],
                                               ALU.mult, ALU.add)
                nc.vector.scalar_tensor_tensor(AE[:], cv3e[:, :, 0], A1b, TMP[:],
                                               ALU.mult, ALU.add)
                nc.vector.tensor_add(AE[:], AE[:], EPN[:])
                nc.vector.tensor_scalar(TMP[:], cv3e[:, :, 5], B2b, None, ALU.mult)
                nc.vector.scalar_tensor_tensor(BE[:], cv3e[:, :, 1], B1b, TMP[:],
                                               ALU.mult, ALU.add)
                nc.vector.tensor_scalar_mul(DE[:], cv3e[:, :, 4], float(SXY * SXY))
                nc.vector.tensor_add(DE[:], DE[:], EPN[:])

                DET = sb.tile([128, B], f32, tag=tg("DET"))
                nc.vector.tensor_mul(DET[:], AE[:], DE[:])
                nc.vector.tensor_mul(TMP[:], BE[:], BE[:])
                nc.vector.tensor_sub(DET[:], DET[:], TMP[:])
                IDN = sb.tile([128, B], f32, tag=tg("IDN"))
                nc.vector.reciprocal(IDN[:], DET[:])
                nc.vector.tensor_mul(IDN[:], IDN[:], NSQ[:])
                IA = sb.tile([128, B], f32, tag=tg("IA"))
                ID_ = sb.tile([128, B], f32, tag=tg("ID_"))
                F2T = sb.tile([128, B], f32, tag=tg("F2T"))
                nc.vector.tensor_mul(IA[:], DE[:], IDN[:])
                nc.vector.tensor_mul(F2T[:], BE[:], IDN[:])
                nc.vector.tensor_mul(ID_[:], AE[:], IDN[:])

                F6 = sb.tile([128, B * 6], f32, tag=tg("F6"))
                f63 = F6[:].rearrange("p (b k) -> p b k", k=6)
                nc.vector.tensor_scalar_mul(f63[:, :, 0], IA[:], -0.5)
                nc.vector.tensor_copy(f63[:, :, 1], F2T[:])
                nc.vector.tensor_scalar_mul(f63[:, :, 2], ID_[:], -0.5)
                nc.vector.tensor_mul(TMP[:], IA[:], MX[:])
                nc.vector.tensor_mul(TMP2[:], F2T[:], MY[:])
                nc.vector.tensor_sub(f63[:, :, 3], TMP[:], TMP2[:])
                nc.vector.tensor_mul(TMP[:], ID_[:], MY[:])
                nc.vector.tensor_mul(TMP2[:], F2T[:], MX[:])
                nc.vector.tensor_sub(f63[:, :, 4], TMP[:], TMP2[:])
                nc.vector.tensor_mul(TMP[:], MX[:], f63[:, :, 3])
                nc.vector.tensor_mul(TMP2[:], MY[:], f63[:, :, 4])
                nc.vector.tensor_add(TMP[:], TMP[:], TMP2[:])
                nc.vector.tensor_mul(TMP2[:], Z3[:], TD[:])
                nc.vector.tensor_add(TMP[:], TMP[:], TMP2[:])
                nc.vector.tensor_scalar_mul(f63[:, :, 5], TMP[:], -0.5)

                # F18 bf16 slots: s0 = f0, s1 = f0 (pairs with g1), s2 = f1
                F18 = sb.tile([128, B * KP], bf16, tag=tg("F18"))
                f364 = F18[:].rearrange("p (b s k) -> p b s k", s=NSLOT, k=6)
                R1 = sb.tile([128, B * 6], f32, tag=tg("R1"))
                r13 = R1[:].rearrange("p (b k) -> p b k", k=6)
                nc.gpsimd.tensor_copy(f364[:, :, 0, :], f63)
                nc.gpsimd.tensor_copy(f364[:, :, 1, :], f364[:, :, 0, :])
                nc.vector.tensor_sub(r13, f63, f364[:, :, 0, :])
                nc.gpsimd.tensor_copy(f364[:, :, 2, :], r13)
                return F18

            F18s = [None, None]
            F18s[0] = preprocess_half(0)
            # FS[h]: [128, 512] bf16; partition group i rows 32i..32i+17 hold
            # F^T for the half's i-th 4-block chunk (512 gaussians).
            FS = [sb.tile([128, 512], bf16, tag=f"FS{h}", name=f"FS{h}")
                  for h in range(2)]

            ACC = sb.tile([128, 2 * ROWS], f32)
            IMG = sb.tile([128, ROWS], f32)
            RMX = sb.tile([128, 1], f32)

            def transpose_half(h):
                TP = dp.tile([128, 512], bf16, tag="pt")
                for i in range(4):
                    for c in range(4):
                        b = 4 * i + c
                        nc.tensor.transpose(
                            TP[32 * i:32 * i + KP, 128 * c:128 * (c + 1)],
                            F18s[h][:, KP * b:KP * (b + 1)], IDB[:],
                            tile_position=(0, 32 * i))
                nc.vector.tensor_copy(FS[h][:], TP[:])

            def dense_sweep(r, h):
                PT = dp.tile([128, 2048], dt.float32, tag="pt")
                for i in range(4):
                    nc.tensor.matmul(
                        PT[:, 512 * i:512 * (i + 1)],
                        G_SB[32 * i:32 * i + KP, 128 * r:128 * (r + 1)],
                        FS[h][32 * i:32 * i + KP, :],
                        start=True, stop=True,
                        tile_position=(32 * i, 0))
                col = 2 * r + h
                nc.scalar.activation(PT[:], PT[:], AF.Exp,
                                     accum_out=ACC[:, col:col + 1])

            with tc.tile_pool(name="dp", bufs=2, space="PSUM") as dp:
                transpose_half(0)
                F18s[1] = preprocess_half(1)
                for r in range(ROWS):
                    dense_sweep(r, 0)
                    if r == 12:
                        transpose_half(1)
                for r in range(ROWS):
                    dense_sweep(r, 1)

            acc3 = ACC[:].rearrange("p (r h) -> p r h", h=2)
            nc.vector.tensor_add(IMG[:], acc3[:, :, 0], acc3[:, :, 1])
            nc.vector.reduce_max(RMX[:], IMG[:], axis=mybir.AxisListType.X)

            # ------- phase 3: global max (AllGather) + normalize -------
            with tc.tile_pool(name="tp", bufs=1, space="PSUM") as tp:
                RMTp = tp.tile([1, 128], dt.float32)
                nc.tensor.transpose(RMTp[:], RMX[:], IDF[:])
                RMT = sb.tile([1, 128], dt.float32)
                nc.vector.tensor_copy(RMT[:], RMTp[:])
                LMAX = sb.tile([1, 1], dt.float32)
                nc.vector.reduce_max(LMAX[:], RMT[:], axis=mybir.AxisListType.X)
                cin = dram.tile([1, 1], dt.float32)
                cout = dram.tile([NCORES, 1], dt.float32)
                nc.sync.dma_start(cin[:], LMAX[:])

                # transpose the unnormalized image while the collective runs
                OTp = tp.tile([ROWS, 128], dt.float32)
                nc.tensor.transpose(OTp[:], IMG[:], IDF[:])
                OT = sb.tile([ROWS, 128], dt.float32)
                nc.vector.tensor_copy(OT[:], OTp[:])

                nc.gpsimd.collective_compute(
                    "AllGather", ALU.bypass,
                    replica_groups=[list(range(NCORES))],
                    ins=[cin[:].opt()], outs=[cout[:].opt()])
                GM8 = sb.tile([1, NCORES], dt.float32)
                nc.sync.dma_start(GM8[:], cout[:].rearrange("p q -> q p"))
                GM = sb.tile([1, 1], dt.float32)
                nc.vector.reduce_max(GM[:], GM8[:], axis=mybir.AxisListType.X)
                nc.vector.tensor_scalar_max(GM[:], GM[:], float(EPS))
                RI = sb.tile([1, 1], dt.float32)
                nc.vector.reciprocal(RI[:], GM[:])
                RIBp = tp.tile([ROWS, 1], dt.float32)
                nc.tensor.matmul(RIBp[:], ONES[:, 0:ROWS], RI[:],
                                 start=True, stop=True)
                RIB = sb.tile([ROWS, 1], dt.float32)
                nc.vector.tensor_copy(RIB[:], RIBp[:])
                nc.vector.tensor_scalar(OT[:], OT[:], RIB[:], None, ALU.mult)
                nc.sync.dma_start(out_t[:], OT[:])

    nc.compile()
    return nc


_NC_CACHE = {}


def _get_nc():
    if "nc" not in _NC_CACHE:
        _NC_CACHE["nc"] = build_nc()
    return _NC_CACHE["nc"]


def _make_in_maps(means, raw_scales, rotors, t, angle):
    means = np.ascontiguousarray(np.asarray(means, np.float32))
    raw_scales = np.ascontiguousarray(np.asarray(raw_scales, np.float32))
    rotors = np.ascontiguousarray(np.asarray(rotors, np.float32))
    t_arr = np.array([[np.float32(t)]], np.float32)
    a_arr = np.array([[np.float32(angle)]], np.float32)
    idb = np.eye(128, dtype=np.float32).astype(ml_dtypes.bfloat16)
    idf = np.eye(128, dtype=np.float32)
    ones = np.ones((1, 128), np.float32)
    in_maps = []
    for c in range(NCORES):
        in_maps.append({
            "means": means, "raw_scales": raw_scales, "rotors": rotors,
            "t_scalar": t_arr, "angle": a_arr,
            "g_lhsT": np.ascontiguousarray(_g_lhsT_for_core(c)),
            "ident_bf": idb, "ident_f32": idf, "ones_row": ones,
        })
    return in_maps


def run(means, raw_scales, rotors, t, angle, trace=False):
    """Returns (image [128,128] fp32, BassKernelResults)."""
    nc = _get_nc()
    in_maps = _make_in_maps(means, raw_scales, rotors, t, angle)
    res = bass_utils.run_bass_kernel_spmd(
        nc, in_maps, core_ids=list(range(NCORES)), trace=trace)
    img = np.concatenate([res.results[c]["out"] for c in range(NCORES)], axis=0)
    return img.astype(np.float32), res


def kernel(**inputs):
    img, _ = run(inputs["means"], inputs["raw_scales"], inputs["rotors"],
                 inputs["t"], inputs["angle"])
    return img


# revision 12
# speedup vs baseline: 1.1482x; 1.1428x over previous
"""Trainium2 Bass kernel for nn_BatchedGaussianRenderer.

Math: each gaussian's per-pixel exponent is expanded as a 6-term polynomial
in centered pixel coordinates (x', y') = (x-63.5, y-63.5):

  expo(n, x, y) = f1*x'^2 + f2*x'y' + f3*y'^2 + f4*x' + f5*y' + f6
  image(x, y)   = sum_n exp(expo(n, x, y)),  then / max(image)

so the dense N x P evaluation is a K=6 matmul.  For accuracy on the bf16
tensor engine, f and the pixel basis g are each split into 2 bf16
components and the 3 leading cross products kept (K=18, error ~2^-17 per
term, validated at ~3e-6 absmax-rel vs an fp64 oracle).

Sharding: each of the 8 cores computes ALL 4096 gaussians' coefficients
(cheap, ~100 vector ops on [128, blocks] layouts) and renders 16 image
rows (pixels x on partitions, gaussians streamed).  K=18 <= 32 lets the
dense matmuls be packed 4-to-the-PE-array via tile_position row groups:
per image row, 4 concurrent [18x128]x[18x512] matmuls cover 2048
gaussians in ~512 PE cycles.  The ScalarEngine's fused exp+row-sum
(accum_out) produces the image directly; an 8-value AllGather + local max
replaces the AllReduce for the final normalization.

The gaussians are processed in two halves so the second half's coefficient
computation (VectorEngine) overlaps the first half's rendering (ScalarE).

Per-gaussian preprocessing uses the unnormalized-quaternion fold: cov4D
scales uniformly by nsq = |q1|^2 |q2|^2, which cancels everywhere except
eps -> eps*nsq and inv_cov/lambda -> *nsq, avoiding rsqrt entirely.
sin/cos of the view angle are evaluated as Taylor polynomials on the DVE
(angle in [0,1)) so the only ACT table set ever loaded is exp's.
"""
import numpy as np
import ml_dtypes

import concourse.bass as bass
import concourse.bacc as bacc
import concourse.tile as tile
import concourse.mybir as mybir
from concourse import bass_utils

NG, H, W = 4096, 128, 128
ZOOM, EPS = 0.5, 1e-6
CX = CY = 63.5
SXY = (W - 1) / 2 * ZOOM          # 31.75
NCORES = 8
ROWS = H // NCORES                # 16 image rows per core
NB = NG // 128                    # 32 gaussian blocks (g = p*NB + b)
HB = NB // 2                      # 16 blocks per half
NSLOT = 3                         # (f-split, g-split) pairs: 00, 01, 10
KP = NSLOT * 6                    # 18 K rows
dt = mybir.dt
AF = mybir.ActivationFunctionType
ALU = mybir.AluOpType

# ---------------------------------------------------------------- host helpers

def _bf16(x):
    return np.asarray(x, np.float32).astype(ml_dtypes.bfloat16).astype(np.float32)


def _g_lhsT_for_core(core):
    """[128, ROWS*128] bf16 pixel-basis weights, replicated into the four
    32-partition groups (rows 32i..32i+17 identical) for tile_position
    row-group packing; see module docstring."""
    jg_of_s = (0, 1, 0)           # g-split component per slot
    out = np.zeros((128, ROWS * 128), np.float32)
    x = np.arange(128, dtype=np.float64) - CX
    for r in range(ROWS):
        y = ROWS * core + r - CY
        basis = np.stack([x * x, x * y, np.full(128, y * y), x,
                          np.full(128, y), np.ones(128)], 0)
        b32 = basis.astype(np.float32)
        g0 = _bf16(b32)
        g1 = _bf16(b32 - g0)
        gs = (g0, g1)
        for s in range(NSLOT):
            for k in range(6):
                row = gs[jg_of_s[s]][k]
                for i in range(4):
                    out[32 * i + s * 6 + k, 128 * r:128 * (r + 1)] = row
    return out.astype(ml_dtypes.bfloat16)


# L(q1) twisted copies: (out_off, out_stride, in_off, in_stride, count, sign),
# L stored per-block (i,k) slot = i*4+k, rotor comps a=(r0,r4,r5,r6).
L_COPIES = [
    (0, 1, 0, 1, 1, 1.0), (4, 4, 4, 1, 3, 1.0),
    (1, 12, 4, 1, 2, -1.0), (5, 4, 0, 6, 2, 1.0),
    (2, 4, 5, 1, 2, -1.0), (10, 4, 0, 4, 2, 1.0),
    (3, 1, 6, 1, 1, -1.0), (11, 1, 4, 1, 1, -1.0),
    (7, 1, 5, 1, 1, 1.0), (15, 1, 0, 1, 1, 1.0),
]
# R(conj q2) with q2 = (r7, -r1, -r2, -r3) folded; stored (j,k) slot = j*4+k.
R_COPIES = [
    (0, 1, 7, 1, 1, 1.0), (4, 4, 1, 1, 3, -1.0),
    (1, 4, 1, 6, 2, 1.0), (9, 1, 3, 1, 1, -1.0), (13, 1, 2, 1, 1, 1.0),
    (2, 4, 2, 1, 2, 1.0), (10, 1, 7, 1, 1, 1.0), (14, 1, 1, 1, 1, -1.0),
    (3, 1, 3, 1, 1, 1.0), (7, 1, 2, 1, 1, -1.0), (11, 4, 1, 6, 2, 1.0),
]

SIN_C = [1.0, -1.0 / 6, 1.0 / 120, -1.0 / 5040, 1.0 / 362880]      # of x^(2k+1)
COS_C = [1.0, -0.5, 1.0 / 24, -1.0 / 720, 1.0 / 40320, -1.0 / 3628800]


def build_nc():
    nc = bacc.Bacc("TRN2", target_bir_lowering=False, debug=False,
                   num_devices=NCORES)
    f32, bf16 = dt.float32, dt.bfloat16

    # fused inputs: one DMA per dtype class (9 serial DMAs cost ~6us of startup)
    fusf_in = nc.dram_tensor("fused_f32", [128, 640], f32, kind="ExternalInput").ap()
    fusb_in = nc.dram_tensor("fused_bf16", [128, ROWS * 128 + 128], bf16,
                             kind="ExternalInput").ap()
    scal_in = nc.dram_tensor("fused_scal", [1, 131], f32, kind="ExternalInput").ap()
    out_t = nc.dram_tensor("out", [ROWS, W], f32, kind="ExternalOutput").ap()

    with tile.TileContext(nc) as tc:
        with (
            tc.tile_pool(name="sb", bufs=1) as sb,
            tc.tile_pool(name="dram", bufs=1, space="DRAM") as dram,
        ):
            # ---------------- phase 0: loads + angle scalars ----------------
            FUSF = sb.tile([128, 640], f32)
            FUSB = sb.tile([128, ROWS * 128 + 128], bf16)
            SCAL = sb.tile([1, 131], f32)
            nc.sync.dma_start(FUSF[:], fusf_in[:])
            nc.sync.dma_start(SCAL[:], scal_in[:])
            nc.sync.dma_start(FUSB[:], fusb_in[:])
            MEANS = FUSF[:, 0:128]
            RAWS = FUSF[:, 128:256]
            ROT = FUSF[:, 256:512]
            IDF = FUSF[:, 512:640]
            G_SB = FUSB[:, 0:ROWS * 128]
            IDB = FUSB[:, ROWS * 128:ROWS * 128 + 128]
            T_A = SCAL[:, 0:1]
            ANG = SCAL[:, 1:2]
            ONES = SCAL[:, 3:131]

            # sin/cos via Taylor on DVE (angle in [0,1); no trig table load)
            U = sb.tile([1, 1], f32)
            SINA = sb.tile([1, 1], f32)
            COSA = sb.tile([1, 1], f32)
            nc.vector.tensor_mul(U[:], ANG, ANG)
            # cos/sin: Horner in u = x^2 (acc = acc*u + c per step)
            nc.vector.tensor_scalar(COSA[:], U[:], COS_C[5], COS_C[4],
                                    ALU.mult, ALU.add)
            for c in (COS_C[3], COS_C[2], COS_C[1], COS_C[0]):
                nc.vector.tensor_scalar(COSA[:], COSA[:], U[:], c,
                                        ALU.mult, ALU.add)
            nc.vector.tensor_scalar(SINA[:], U[:], SIN_C[4], SIN_C[3],
                                    ALU.mult, ALU.add)
            for c in (SIN_C[2], SIN_C[1], SIN_C[0]):
                nc.vector.tensor_scalar(SINA[:], SINA[:], U[:], c,
                                        ALU.mult, ALU.add)
            nc.vector.tensor_mul(SINA[:], SINA[:], ANG)

            # scalar vector: [sxc, sxs, A1, A2, A3, B1, B2, t]
            SCV = sb.tile([1, 8], f32)
            nc.vector.tensor_scalar_mul(SCV[:, 0:1], COSA[:], float(SXY))
            nc.vector.tensor_scalar_mul(SCV[:, 1:2], SINA[:], float(SXY))
            nc.vector.tensor_mul(SCV[:, 2:3], SCV[:, 0:1], SCV[:, 0:1])
            nc.vector.scalar_tensor_tensor(SCV[:, 3:4], SCV[:, 0:1], 2.0,
                                           SCV[:, 1:2], ALU.mult, ALU.mult)
            nc.vector.tensor_mul(SCV[:, 4:5], SCV[:, 1:2], SCV[:, 1:2])
            nc.vector.tensor_scalar_mul(SCV[:, 5:6], SCV[:, 0:1], float(SXY))
            nc.vector.tensor_scalar_mul(SCV[:, 6:7], SCV[:, 1:2], float(SXY))
            nc.vector.tensor_copy(SCV[:, 7:8], T_A)
            with tc.tile_pool(name="pp0", bufs=1, space="PSUM") as pp0:
                PBp = pp0.tile([128, 8], f32)
                nc.tensor.matmul(PBp[:], ONES, SCV[:], start=True, stop=True)
                SCB = sb.tile([128, 8], f32)
                nc.vector.tensor_copy(SCB[:], PBp[:])
            sxc_b, sxs_b = SCB[:, 0:1], SCB[:, 1:2]
            A1b, A2b, A3b = SCB[:, 2:3], SCB[:, 3:4], SCB[:, 4:5]
            B1b, B2b, tb = SCB[:, 5:6], SCB[:, 6:7], SCB[:, 7:8]

            # warm the CC channel early: the first collective in a NEFF pays
            # tens of us of one-time setup; run a junk AllGather overlapped
            # with preprocessing/dense so the real one at the tail is cheap.
            cinw = dram.tile([1, 1], f32)
            coutw = dram.tile([NCORES, 1], f32)
            nc.sync.dma_start(cinw[:], SCV[:, 0:1])
            nc.gpsimd.collective_compute(
                "AllGather", ALU.bypass,
                replica_groups=[list(range(NCORES))],
                ins=[cinw[:].opt()], outs=[coutw[:].opt()])

            means_a = MEANS.rearrange("p (b c) -> p b c", c=4)
            rot_a = ROT.rearrange("p (b c) -> p b c", c=8)

            def preprocess_half(h):
                """Emit coefficient computation for blocks [HB*h, HB*(h+1)).
                Returns the F18 tile [128, HB*18] bf16 (b, s, k)."""
                tg = lambda n: f"{n}{h}"
                B = HB
                means_h = means_a[:, HB * h:HB * (h + 1), :]
                rot_h = rot_a[:, HB * h:HB * (h + 1), :]
                raws_h = RAWS[:, HB * 4 * h:HB * 4 * (h + 1)]

                S2 = sb.tile([128, B * 4], f32, tag=tg("S2"))
                nc.scalar.activation(S2[:], raws_h, AF.Exp, scale=2.0)

                SQ = sb.tile([128, B * 8], f32, tag=tg("SQ"))
                nc.vector.tensor_mul(SQ[:], rot_h, rot_h)
                sq = SQ[:].rearrange("p (b c) -> p b c", c=8)
                N1S = sb.tile([128, B], f32, tag=tg("N1S"))
                N2S = sb.tile([128, B], f32, tag=tg("N2S"))
                NSQ = sb.tile([128, B], f32, tag=tg("NSQ"))
                nc.vector.reduce_sum(N1S[:], sq[:, :, 4:7], axis=mybir.AxisListType.X)
                nc.vector.tensor_add(N1S[:], N1S[:], sq[:, :, 0])
                nc.vector.reduce_sum(N2S[:], sq[:, :, 1:4], axis=mybir.AxisListType.X)
                nc.vector.tensor_add(N2S[:], N2S[:], sq[:, :, 7])
                nc.vector.tensor_mul(NSQ[:], N1S[:], N2S[:])

                LT = sb.tile([128, B * 16], f32, tag=tg("LT"))
                RT = sb.tile([128, B * 16], f32, tag=tg("RT"))
                lt3 = LT[:].rearrange("p (b c) -> p b c", c=16)
                rt3 = RT[:].rearrange("p (b c) -> p b c", c=16)
                # L copies on DVE, R copies on GpSimd: halves the serial
                # copy latency ahead of the P64 product.
                for dst, copies, eng in ((lt3, L_COPIES, nc.vector),
                                         (rt3, R_COPIES, nc.gpsimd)):
                    for (oo, os_, io, is_, cnt, sign) in copies:
                        out_ap = dst[:, :, oo::os_][:, :, :cnt] if cnt > 1 else dst[:, :, oo:oo + 1]
                        in_ap = rot_h[:, :, io::is_][:, :, :cnt] if cnt > 1 else rot_h[:, :, io:io + 1]
                        if sign > 0:
                            eng.tensor_copy(out_ap, in_ap)
                        else:
                            eng.tensor_scalar_mul(out_ap, in_ap, -1.0)

                P64 = sb.tile([128, B * 64], f32, tag=tg("P64"))
                lt4 = LT[:].rearrange("p (b i k) -> p b i k", i=4, k=4)
                rt4 = RT[:].rearrange("p (b j k) -> p b j k", j=4, k=4)
                p5 = P64[:].rearrange("p (b i j k) -> p b i j k", i=4, j=4, k=4)
                nc.vector.tensor_mul(
                    p5,
                    lt4.unsqueeze(3).broadcast_to([128, B, 4, 4, 4]),
                    rt4.unsqueeze(2).broadcast_to([128, B, 4, 4, 4]))
                R4 = sb.tile([128, B * 16], f32, tag=tg("R4"))
                nc.vector.reduce_sum(
                    R4[:], P64[:].rearrange("p (e k) -> p e k", k=4),
                    axis=mybir.AxisListType.X)

                M = sb.tile([128, B * 16], f32, tag=tg("M"))
                r44 = R4[:].rearrange("p (b i j) -> p b i j", i=4, j=4)
                s23 = S2[:].rearrange("p (b c) -> p b c", c=4)
                m4 = M[:].rearrange("p (b i j) -> p b i j", i=4, j=4)
                nc.vector.tensor_mul(
                    m4, r44, s23.unsqueeze(2).broadcast_to([128, B, 4, 4]))
                C64 = sb.tile([128, B * 64], f32, tag=tg("C64"))
                c5 = C64[:].rearrange("p (b i k j) -> p b i k j", i=4, k=4, j=4)
                nc.vector.tensor_mul(
                    c5,
                    m4.unsqueeze(3).broadcast_to([128, B, 4, 4, 4]),
                    r44.unsqueeze(2).broadcast_to([128, B, 4, 4, 4]))
                C16 = sb.tile([128, B * 16], f32, tag=tg("C16"))
                nc.vector.reduce_sum(
                    C16[:], C64[:].rearrange("p (e j) -> p e j", j=4),
                    axis=mybir.AxisListType.X)
                c16 = C16[:].rearrange("p (b e) -> p b e", e=16)

                EPN = sb.tile([128, B], f32, tag=tg("EPN"))
                nc.vector.tensor_scalar_mul(EPN[:], NSQ[:], float(EPS))
                WP = sb.tile([128, B], f32, tag=tg("WP"))
                nc.vector.tensor_max(WP[:], c16[:, :, 15], EPN[:])
                IW = sb.tile([128, B], f32, tag=tg("IW"))
                nc.vector.reciprocal(IW[:], WP[:])
                TD = sb.tile([128, B], f32, tag=tg("TD"))
                nc.scalar.activation(TD[:], means_h[:, :, 3], AF.Identity,
                                     bias=tb, scale=-1.0)
                TDW = sb.tile([128, B], f32, tag=tg("TDW"))
                nc.vector.tensor_mul(TDW[:], TD[:], IW[:])
                W1 = sb.tile([128, B], f32, tag=tg("W1"))
                nc.vector.tensor_mul(W1[:], NSQ[:], IW[:])
                Z3 = sb.tile([128, B], f32, tag=tg("Z3"))
                nc.vector.tensor_mul(Z3[:], W1[:], TD[:])

                VV9 = sb.tile([128, B * 9], f32, tag=tg("VV9"))
                vv3 = VV9[:].rearrange("p (b i k) -> p b i k", i=3, k=3)
                v_i = c16[:, :, 3::4][:, :, 0:3]
                nc.vector.tensor_mul(
                    vv3,
                    v_i.unsqueeze(3).broadcast_to([128, B, 3, 3]),
                    v_i.unsqueeze(2).broadcast_to([128, B, 3, 3]))
                CV3 = sb.tile([128, B * 9], f32, tag=tg("CV3"))
                cv3f = CV3[:].rearrange("p (b e) -> p b e", e=9)
                iw_b9 = IW[:].unsqueeze(2).broadcast_to([128, B, 9])
                vv9f = VV9[:].rearrange("p (b e) -> p b e", e=9)
                nc.vector.tensor_mul(cv3f, vv9f, iw_b9)
                u9 = c16.rearrange("p b (i k) -> p b i k", i=4)[:, :, 0:3, 0:3]
                cv33 = CV3[:].rearrange("p (b i k) -> p b i k", i=3, k=3)
                nc.vector.tensor_sub(cv33, u9, cv33)

                MU3 = sb.tile([128, B * 3], f32, tag=tg("MU3"))
                mu33 = MU3[:].rearrange("p (b c) -> p b c", c=3)
                tdw_b3 = TDW[:].unsqueeze(2).broadcast_to([128, B, 3])
                nc.vector.tensor_mul(mu33, v_i, tdw_b3)
                nc.vector.tensor_add(mu33, mu33, means_h[:, :, 0:3])

                MX = sb.tile([128, B], f32, tag=tg("MX"))
                MY = sb.tile([128, B], f32, tag=tg("MY"))
                TMP = sb.tile([128, B], f32, tag=tg("TMP"))
                TMP2 = sb.tile([128, B], f32, tag=tg("TMP2"))
                nc.vector.tensor_scalar(TMP[:], mu33[:, :, 2], sxs_b, None, ALU.mult)
                nc.vector.scalar_tensor_tensor(MX[:], mu33[:, :, 0], sxc_b, TMP[:],
                                               ALU.mult, ALU.add)
                nc.vector.tensor_scalar_mul(MY[:], mu33[:, :, 1], float(SXY))

                cv3e = CV3[:].rearrange("p (b e) -> p b e", e=9)
                AE = sb.tile([128, B], f32, tag=tg("AE"))
                BE = sb.tile([128, B], f32, tag=tg("BE"))
                DE = sb.tile([128, B], f32, tag=tg("DE"))
                nc.vector.tensor_scalar(TMP[:], cv3e[:, :, 8], A3b, None, ALU.mult)
                nc.vector.scalar_tensor_tensor(TMP[:], cv3e[:, :, 2], A2b, TMP[:],
                                               ALU.mult, ALU.add)
                nc.vector.scalar_tensor_tensor(AE[:], cv3e[:, :, 0], A1b, TMP[:],
                                               ALU.mult, ALU.add)
                nc.vector.tensor_add(AE[:], AE[:], EPN[:])
                nc.vector.tensor_scalar(TMP[:], cv3e[:, :, 5], B2b, None, ALU.mult)
                nc.vector.scalar_tensor_tensor(BE[:], cv3e[:, :, 1], B1b, TMP[:],
                                               ALU.mult, ALU.add)
                nc.vector.tensor_scalar_mul(DE[:], cv3e[:, :, 4], float(SXY * SXY))
                nc.vector.tensor_add(DE[:], DE[:], EPN[:])

                DET = sb.tile([128, B], f32, tag=tg("DET"))
                nc.vector.tensor_mul(DET[:], AE[:], DE[:])
                nc.vector.tensor_mul(TMP[:], BE[:], BE[:])
                nc.vector.tensor_sub(DET[:], DET[:], TMP[:])
                IDN = sb.tile([128, B], f32, tag=tg("IDN"))
                nc.vector.reciprocal(IDN[:], DET[:])
                nc.vector.tensor_mul(IDN[:], IDN[:], NSQ[:])
                IA = sb.tile([128, B], f32, tag=tg("IA"))
                ID_ = sb.tile([128, B], f32, tag=tg("ID_"))
                F2T = sb.tile([128, B], f32, tag=tg("F2T"))
                nc.vector.tensor_mul(IA[:], DE[:], IDN[:])
                nc.vector.tensor_mul(F2T[:], BE[:], IDN[:])
                nc.vector.tensor_mul(ID_[:], AE[:], IDN[:])

                F6 = sb.tile([128, B * 6], f32, tag=tg("F6"))
                f63 = F6[:].rearrange("p (b k) -> p b k", k=6)
                nc.vector.tensor_scalar_mul(f63[:, :, 0], IA[:], -0.5)
                nc.vector.tensor_copy(f63[:, :, 1], F2T[:])
                nc.vector.tensor_scalar_mul(f63[:, :, 2], ID_[:], -0.5)
                nc.vector.tensor_mul(TMP[:], IA[:], MX[:])
                nc.vector.tensor_mul(TMP2[:], F2T[:], MY[:])
                nc.vector.tensor_sub(f63[:, :, 3], TMP[:], TMP2[:])
                nc.vector.tensor_mul(TMP[:], ID_[:], MY[:])
                nc.vector.tensor_mul(TMP2[:], F2T[:], MX[:])
                nc.vector.tensor_sub(f63[:, :, 4], TMP[:], TMP2[:])
                nc.vector.tensor_mul(TMP[:], MX[:], f63[:, :, 3])
                nc.vector.tensor_mul(TMP2[:], MY[:], f63[:, :, 4])
                nc.vector.tensor_add(TMP[:], TMP[:], TMP2[:])
                nc.vector.tensor_mul(TMP2[:], Z3[:], TD[:])
                nc.vector.tensor_add(TMP[:], TMP[:], TMP2[:])
                nc.vector.tensor_scalar_mul(f63[:, :, 5], TMP[:], -0.5)

                # F18 bf16 slots: s0 = f0, s1 = f0 (pairs with g1), s2 = f1
                F18 = sb.tile([128, B * KP], bf16, tag=tg("F18"))
                f364 = F18[:].rearrange("p (b s k) -> p b s k", s=NSLOT, k=6)
                R1 = sb.tile([128, B * 6], f32, tag=tg("R1"))
                r13 = R1[:].rearrange("p (b k) -> p b k", k=6)
                nc.gpsimd.tensor_copy(f364[:, :, 0, :], f63)
                nc.gpsimd.tensor_copy(f364[:, :, 1, :], f364[:, :, 0, :])
                nc.vector.tensor_sub(r13, f63, f364[:, :, 0, :])
                nc.gpsimd.tensor_copy(f364[:, :, 2, :], r13)
                return F18

            F18s = [None, None]
            F18s[0] = preprocess_half(0)
            # FS[h]: [128, 512] bf16; partition group i rows 32i..32i+17 hold
            # F^T for the half's i-th 4-block chunk (512 gaussians).
            FS = [sb.tile([128, 512], bf16, tag=f"FS{h}", name=f"FS{h}")
                  for h in range(2)]

            ACC = sb.tile([128, 2 * ROWS], f32)
            IMG = sb.tile([128, ROWS], f32)
            RMX = sb.tile([128, 1], f32)

            def transpose_half(h):
                TP = dp.tile([128, 512], bf16, tag="pt")
                for i in range(4):
                    for c in range(4):
                        b = 4 * i + c
                        nc.tensor.transpose(
                            TP[32 * i:32 * i + KP, 128 * c:128 * (c + 1)],
                            F18s[h][:, KP * b:KP * (b + 1)], IDB,
                            tile_position=(0, 32 * i))
                nc.vector.tensor_copy(FS[h][:], TP[:])

            def dense_sweep(r, h):
                PT = dp.tile([128, 2048], dt.float32, tag="pt")
                for i in range(4):
                    nc.tensor.matmul(
                        PT[:, 512 * i:512 * (i + 1)],
                        G_SB[32 * i:32 * i + KP, 128 * r:128 * (r + 1)],
                        FS[h][32 * i:32 * i + KP, :],
                        start=True, stop=True,
                        tile_position=(32 * i, 0))
                col = 2 * r + h
                nc.scalar.activation(PT[:], PT[:], AF.Exp,
                                     accum_out=ACC[:, col:col + 1])

            with tc.tile_pool(name="dp", bufs=2, space="PSUM") as dp:
                transpose_half(0)
                for r in range(ROWS):
                    dense_sweep(r, 0)
                    if r == 0:
                        # h1 preprocessing fills the DVE/GpSimd queues behind
                        # h0's work; it overlaps the h0 dense phase.
                        F18s[1] = preprocess_half(1)
                    if r == 12:
                        transpose_half(1)
                for r in range(ROWS):
                    dense_sweep(r, 1)

            acc3 = ACC[:].rearrange("p (r h) -> p r h", h=2)
            nc.vector.tensor_add(IMG[:], acc3[:, :, 0], acc3[:, :, 1])
            nc.vector.reduce_max(RMX[:], IMG[:], axis=mybir.AxisListType.X)

            # ------- phase 3: global max (AllGather) + normalize -------
            with tc.tile_pool(name="tp", bufs=1, space="PSUM") as tp:
                RMTp = tp.tile([1, 128], dt.float32)
                nc.tensor.transpose(RMTp[:], RMX[:], IDF)
                RMT = sb.tile([1, 128], dt.float32)
                nc.vector.tensor_copy(RMT[:], RMTp[:])
                LMAX = sb.tile([1, 1], dt.float32)
                nc.vector.reduce_max(LMAX[:], RMT[:], axis=mybir.AxisListType.X)
                cin = dram.tile([1, 1], dt.float32)
                cout = dram.tile([NCORES, 1], dt.float32)
                nc.sync.dma_start(cin[:], LMAX[:])

                # transpose the unnormalized image while the collective runs
                OTp = tp.tile([ROWS, 128], dt.float32)
                nc.tensor.transpose(OTp[:], IMG[:], IDF)
                OT = sb.tile([ROWS, 128], dt.float32)
                nc.vector.tensor_copy(OT[:], OTp[:])

                nc.gpsimd.collective_compute(
                    "AllGather", ALU.bypass,
                    replica_groups=[list(range(NCORES))],
                    ins=[cin[:].opt()], outs=[cout[:].opt()])
                GM8 = sb.tile([1, NCORES], dt.float32)
                nc.sync.dma_start(GM8[:], cout[:].rearrange("p q -> q p"))
                GM = sb.tile([1, 1], dt.float32)
                nc.vector.reduce_max(GM[:], GM8[:], axis=mybir.AxisListType.X)
                nc.vector.tensor_scalar_max(GM[:], GM[:], float(EPS))
                RI = sb.tile([1, 1], dt.float32)
                nc.vector.reciprocal(RI[:], GM[:])
                RIBp = tp.tile([ROWS, 1], dt.float32)
                nc.tensor.matmul(RIBp[:], ONES[:, 0:ROWS], RI[:],
                                 start=True, stop=True)
                RIB = sb.tile([ROWS, 1], dt.float32)
                nc.vector.tensor_copy(RIB[:], RIBp[:])
                nc.vector.tensor_scalar(OT[:], OT[:], RIB[:], None, ALU.mult)
                nc.sync.dma_start(out_t[:], OT[:])

    nc.compile()
    return nc


_NC_CACHE = {}


def _get_nc():
    if "nc" not in _NC_CACHE:
        _NC_CACHE["nc"] = build_nc()
    return _NC_CACHE["nc"]


def _make_in_maps(means, raw_scales, rotors, t, angle):
    means = np.asarray(means, np.float32).reshape(128, 128)
    raw_scales = np.asarray(raw_scales, np.float32).reshape(128, 128)
    rotors = np.asarray(rotors, np.float32).reshape(128, 256)
    fusf = np.concatenate(
        [means, raw_scales, rotors, np.eye(128, dtype=np.float32)], axis=1)
    fusf = np.ascontiguousarray(fusf)
    idb = np.eye(128, dtype=np.float32).astype(ml_dtypes.bfloat16)
    scal = np.ones((1, 131), np.float32)
    scal[0, 0] = np.float32(t)
    scal[0, 1] = np.float32(angle)
    in_maps = []
    for c in range(NCORES):
        fusb = np.ascontiguousarray(
            np.concatenate([_g_lhsT_for_core(c), idb], axis=1))
        in_maps.append({
            "fused_f32": fusf, "fused_bf16": fusb, "fused_scal": scal,
        })
    return in_maps


def run(means, raw_scales, rotors, t, angle, trace=False):
    """Returns (image [128,128] fp32, BassKernelResults)."""
    nc = _get_nc()
    in_maps = _make_in_maps(means, raw_scales, rotors, t, angle)
    res = bass_utils.run_bass_kernel_spmd(
        nc, in_maps, core_ids=list(range(NCORES)), trace=trace)
    img = np.concatenate([res.results[c]["out"] for c in range(NCORES)], axis=0)
    return img.astype(np.float32), res


def kernel(**inputs):
    img, _ = run(inputs["means"], inputs["raw_scales"], inputs["rotors"],
                 inputs["t"], inputs["angle"])
    return img
